# revision 14
# baseline (speedup 1.0000x reference)
"""Trainium2 Bass kernel for the e3nn-style 3D convolution problem.

Host side: builds the tiny [3,3,3,32,64] conv kernel from the radial/spherical
weights (replicating the reference math in fp32 numpy), folds the pointwise
self-connection into the center tap, and pre-arranges the input as a z-im2col
(3 z-shifted copies x 32 channels = 96 partitions) padded volume per batch.

Device side (per core, batch-parallel over 8 cores): 3D conv as accumulated
matmuls. Quantization split: everything except the center tap k[1,1,1] runs as
fp8(e4m3) DoubleRow matmuls — each DoubleRow instruction contracts TWO 96-row
windows at once (k-tile 0 = dx-paired taps (kx=0|kx=2) at base plane xi,
k-tile 1 = center-column singles (kx=1) at plane xi+1, same ky window — the
two k-tiles are adjacent planes of a 3D [96, 34, 1088] im2col tile, so the
rhs AP is a plain 3D slice). The dominant center tap (with the folded
self-connection) runs as one small fp16 matmul (K=32) per output tile from a
separate fp16 copy of the unshifted input. 4 matmuls per plane-group/half
instead of 6 full-rate ones; PSUM accumulation with cross-plane deferred
evacuation; output staged+DMAd as fp16 and upcast on host.
"""

import math

import numpy as np
import ml_dtypes

# ---- problem constants (hardcoded; kernel.py must be self-contained) ----
MUL_IN, MUL_OUT = 8, 16
DIM_IN, DIM_OUT = 4 * MUL_IN, 4 * MUL_OUT  # 32, 64
DIAMETER = 3.0
NUM_RB = 4
BATCH, GRID = 8, 32
N_CORES = 8

XP = GRID + 2  # padded x planes: -1 .. 32
YP = GRID + 2  # padded y rows
PLANE = YP * GRID  # floats per (padded-y, z) plane = 34*32 = 1088
KPART = 3 * DIM_IN  # 96 partitions: z-shift blocks (dz=-1,0,+1) x 32 channels
OUT_COLS = GRID * 512  # out dram [128, 16384]

FP8 = ml_dtypes.float8_e4m3  # TRN FP8_EXP4: bias 7, max normal +-240


# --------------------------------------------------------------------------
# host-side math: replicate the reference kernel build in fp32 numpy
# --------------------------------------------------------------------------
def _sus(x):
    # smooth unit step: exp(-1/x) for x>0 else 0
    safe = np.where(x > 0.0, x, 1.0).astype(np.float32)
    return np.where(x > 0.0, np.exp(np.float32(-1.0) / safe), np.float32(0.0))


def build_conv_kernel(w_lin0, w_lin1, w000, w011, w101, w110):
    """Returns K [3,3,3,DIM_IN,DIM_OUT] fp32 with the self-connection folded
    into the center tap."""
    f32 = np.float32
    r = DIAMETER / 2
    ax = np.arange(-math.floor(r), math.floor(r) + 1.0, dtype=f32)  # [-1,0,1]
    lattice = np.stack(np.meshgrid(ax, ax, ax, indexing="ij"), axis=-1).astype(f32)

    dist = np.linalg.norm(lattice, axis=-1).astype(f32)  # [3,3,3]
    values = np.linspace(0.0, DIAMETER / 2, NUM_RB + 2, dtype=f32)
    step = values[1] - values[0]
    diff = (dist[..., None] - values[1:-1]) / step  # [3,3,3,4]
    emb = (f32(1.14136) * np.exp(f32(2.0)) * _sus(diff + 1.0) * _sus(1.0 - diff)).astype(f32)

    norm = np.linalg.norm(lattice, axis=-1, keepdims=True).astype(f32)
    unit = lattice / np.where(norm == 0.0, f32(1.0), norm)
    sh1 = (np.sqrt(f32(3.0)) * unit).astype(f32)  # [3,3,3,3]

    n_lat = 27

    def rad(w):
        # emb [3,3,3,4] x w [4,8,1,16] -> [3,3,3,8,16]
        return (np.einsum("xyzk,kuvw->xyzuw", emb, w.astype(f32)) / f32(n_lat)).astype(f32)

    r000, r011, r101, r110 = rad(w000), rad(w011), rad(w101), rad(w110)

    inv_s3 = f32(1.0 / math.sqrt(3.0))
    alpha = f32(1.0 / math.sqrt(2.0 * MUL_IN))

    k00 = (alpha * r000).astype(f32)  # [3,3,3,8,16]
    k01 = (alpha * inv_s3) * np.einsum("xyzuw,xyzm->xyzuwm", r011, sh1)
    k01 = k01.reshape(3, 3, 3, MUL_IN, 3 * MUL_OUT).astype(f32)
    k10 = (alpha * inv_s3) * np.einsum("xyzuw,xyzi->xyzuiw", r110, sh1)
    k10 = k10.reshape(3, 3, 3, 3 * MUL_IN, MUL_OUT).astype(f32)
    eye3 = np.eye(3, dtype=f32)
    k11 = (alpha * inv_s3) * np.einsum("xyzuw,im->xyzuiwm", r101, eye3)
    k11 = k11.reshape(3, 3, 3, 3 * MUL_IN, 3 * MUL_OUT).astype(f32)

    k = np.concatenate(
        [
            np.concatenate([k00, k01], axis=-1),
            np.concatenate([k10, k11], axis=-1),
        ],
        axis=-2,
    ).astype(f32)  # [3,3,3,32,64]

    # ---- self-connection folded into the center tap ----
    lin_norm = f32(1.0 / math.sqrt(MUL_IN))
    w_sc = np.zeros((DIM_IN, DIM_OUT), f32)
    w_sc[:MUL_IN, :MUL_OUT] = w_lin0.astype(f32) * lin_norm
    for i in range(3):
        rows = MUL_IN + 3 * np.arange(MUL_IN) + i
        cols = MUL_OUT + 3 * np.arange(MUL_OUT) + i
        w_sc[np.ix_(rows, cols)] = w_lin1.astype(f32) * lin_norm
    k[1, 1, 1] += w_sc
    return k


def q8(a):
    return np.asarray(np.clip(a, -240.0, 240.0), FP8)


def pack_weights(k):
    """[3,3,3,32,64] -> (wk3 [96, 6, 128] fp8, wc3 [128, 2, 64] fp8).

    Contraction row blocks (matching build_im2col): 0-31 dz=-1, 32-63 dz=+1,
    64-95 dz=0.
    wk3[:, 2*ky+0, :]    = [k[0,ky] | k[2,ky]]   (dx-pair, base plane xi)
    wk3[:, 2*ky+1, 0:64] = k[1,ky]               (center column, plane xi+1),
                           with the dz=0 block of ky=1 (the true center tap,
                           incl. folded self-connection) zeroed -> moved to
                           the hi/lo-split center DR weights wc3:
    wc3[64:128, 0] = [W_hi; W_hi], wc3[64:128, 1] = [W_lo; 0]  (x_hi/x_lo
    partition blocks), so the center DR computes
    W_hi*(x_hi+x_lo) + W_lo*x_hi = W*x - W_lo*x_lo.
    """
    perm = np.r_[0:32, 64:96, 32:64]  # (dz-1, dz0, dz+1) -> (dz-1, dz+1, dz0)
    wk3 = np.zeros((KPART, 6, 2 * DIM_OUT), np.float32)
    for ky in range(3):
        wk3[:, 2 * ky, 0:DIM_OUT] = k[0, ky].reshape(KPART, DIM_OUT)[perm]
        wk3[:, 2 * ky, DIM_OUT:] = k[2, ky].reshape(KPART, DIM_OUT)[perm]
        wk3[:, 2 * ky + 1, 0:DIM_OUT] = k[1, ky].reshape(KPART, DIM_OUT)[perm]
    # remove the center tap from the windowed fp8 path (rows 64-95 = dz0)
    wk3[2 * DIM_IN : 3 * DIM_IN, 3, 0:DIM_OUT] = 0.0

    W = k[1, 1, 1].astype(np.float32)
    W_hi = q8(W).astype(np.float32)
    W_lo = W - W_hi
    wc3 = np.zeros((4 * DIM_IN, 2, DIM_OUT), np.float32)
    wc3[2 * DIM_IN : 3 * DIM_IN, 0] = W_hi  # x_hi rows
    wc3[3 * DIM_IN :, 0] = W_hi  # x_lo rows
    wc3[2 * DIM_IN : 3 * DIM_IN, 1] = W_lo
    return q8(wk3), q8(wc3)


def build_im2col(xb):
    """xb [32,32,32,32] (X,Y,Z,C) -> xim [128,34,1088] fp8.

    Block 32*j + c layout: j=0: x[.,.,z-1,c] (dz=-1), j=1: x[.,.,z+1,c]
    (dz=+1), j=2: x_hi = fp8(x) (dz=0), j=3: x_lo = fp8(x - x_hi), all laid
    out as [xp 0..33][yp 0..33][z 0..31] with zero padding at xp/yp borders
    and z-shift edges."""
    xt = np.ascontiguousarray(xb.transpose(3, 0, 1, 2))  # [C, X, Y, Z]
    xim = np.zeros((4 * DIM_IN, XP, YP, GRID), np.float32)
    xim[0:32, 1:33, 1:33, 1:32] = xt[:, :, :, 0:31]  # dz=-1
    xim[32:64, 1:33, 1:33, 0:31] = xt[:, :, :, 1:32]  # dz=+1
    xim[64:96, 1:33, 1:33, :] = xt  # dz=0 -> x_hi
    x8 = q8(xim).reshape(4 * DIM_IN, XP, PLANE)
    lo = xim[64:96] - x8[64:96].astype(np.float32).reshape(DIM_IN, XP, YP, GRID)
    x8[96:128] = q8(lo).reshape(DIM_IN, XP, PLANE)
    return x8


def gather_out(arr):
    """arr [128, 16384] fp16 -> [32, 32, 32, 64] fp32.

    Row p = (h*64 + co); column = xi*512 + yi*32 + z."""
    a = arr.astype(np.float32).reshape(2, DIM_OUT, GRID, 16, GRID)  # [h,co,xi,yi,z]
    return np.ascontiguousarray(a.transpose(2, 0, 3, 4, 1)).reshape(GRID, GRID, GRID, DIM_OUT)


# --------------------------------------------------------------------------
# device program
# --------------------------------------------------------------------------
_PROGRAM_CACHE = {}


def declare_tensors(nc):
    import concourse.mybir as mybir

    return dict(
        xim=nc.dram_tensor("xim", [4 * DIM_IN, XP, PLANE], mybir.dt.float8e4, kind="ExternalInput").ap(),
        wk3=nc.dram_tensor("wk3", [KPART, 6, 2 * DIM_OUT], mybir.dt.float8e4, kind="ExternalInput").ap(),
        wc3=nc.dram_tensor("wc3", [4 * DIM_IN, 2, DIM_OUT], mybir.dt.float8e4, kind="ExternalInput").ap(),
        out=nc.dram_tensor("out", [2 * DIM_OUT, OUT_COLS], mybir.dt.float16, kind="ExternalOutput").ap(),
    )


def build_program():
    import concourse.tile as tile
    from concourse import bacc

    nc = bacc.Bacc(
        "TRN2",
        target_bir_lowering=False,
        debug=False,
        enable_asserts=True,
        num_devices=N_CORES,
    )
    T = declare_tensors(nc)
    with tile.TileContext(nc) as tc:
        emit_body(nc, tc, T)

    nc.compile()
    return nc


def emit_body(nc, tc, T):
    """fp8-DoubleRow scheme with per-plane [128, 512] fp16 output staging.

    For plane group xi (0..32), half h: psum bank (xi, h):
      rows 0-63   = out plane xi   (kx=0 pair-half + kx=1 singles + fp16 center)
      rows 64-127 = out plane xi-2 (kx=2 pair-half)
    Per bank 4 matmuls: DR(ky=0, start) -> center fp16 -> DR(ky=1)
    -> DR(ky=2, stop; full-region LAST so PSUM reads depend on the bank's
    final matmul — partial-region last writers race DVE evac reads on HW).
    Evacuation of plane xi: ob[h*64:(h+1)*64] = bank[xi,h][0:64]
    (+ bank[xi+2,h][64:128]), then one [128, 512] fp16 DMA per plane.
    """
    import concourse.mybir as mybir

    f32 = mybir.dt.float32
    f16 = mybir.dt.float16
    f8 = mybir.dt.float8e4
    DR = mybir.MatmulPerfMode.DoubleRow

    IN_CHUNKS = globals().get("IN_CHUNKS_OVR", 8)
    XC_CHUNKS = globals().get("XC_CHUNKS_OVR", 4)
    ob_bufs = globals().get("OB_BUFS_OVR", 6)
    out_eng = getattr(nc, globals().get("OUT_ENGINE", "scalar"))
    in_eng = getattr(nc, globals().get("IN_ENGINE", "sync"))
    xc_eng = getattr(nc, globals().get("XC_ENGINE", "gpsimd"))

    xim_d, wk3_d, wc3_d, out_d = T["xim"], T["wk3"], T["wc3"], T["out"]
    obdt = f32 if globals().get("OB_DTYPE") == "float32" else f16
    skip_center = globals().get("SKIP_CENTER", False)

    with (
        tc.tile_pool(name="xim", bufs=1) as xim_pool,
        tc.tile_pool(name="wk", bufs=1) as wk_pool,
        tc.tile_pool(name="ob", bufs=ob_bufs) as ob_pool,
        tc.tile_pool(name="ps", bufs=8, space="PSUM") as ps_pool,
    ):
        wk3_t = wk_pool.tile([KPART, 6, 2 * DIM_OUT], f8)
        nc.sync.dma_start(out=wk3_t[...], in_=wk3_d[...])
        wc3_t = wk_pool.tile([4 * DIM_IN, 2, DIM_OUT], f8)
        nc.sync.dma_start(out=wc3_t[...], in_=wc3_d[...])

        xim_t = xim_pool.tile([4 * DIM_IN, XP, PLANE], f8)
        # chunked so matmuls can start once their planes have landed
        # (Tile tracks subtile deps); x_lo block rides on a second queue
        for ci in range(IN_CHUNKS):
            lo = ci * XP // IN_CHUNKS
            hi = (ci + 1) * XP // IN_CHUNKS
            in_eng.dma_start(out=xim_t[0:KPART, lo:hi, :], in_=xim_d[0:KPART, lo:hi, :])
        for ci in range(XC_CHUNKS):
            lo = ci * XP // XC_CHUNKS
            hi = (ci + 1) * XP // XC_CHUNKS
            xc_eng.dma_start(
                out=xim_t[KPART:, lo:hi, :], in_=xim_d[KPART:, lo:hi, :]
            )

        banks = {}
        obs = {}

        def evac(xi, h):
            if xi not in obs:
                obs[xi] = ob_pool.tile([2 * DIM_OUT, 512], obdt, name=f"ob_{xi}", tag="ob")
            ob = obs[xi][h * DIM_OUT : (h + 1) * DIM_OUT, :]
            pa = banks[(xi, h)]
            nc.scalar.copy(ob[:, :], pa[0:DIM_OUT, :])
            if xi < GRID - 1:
                pb = banks[(xi + 2, h)]
                nc.vector.tensor_add(ob[:, :], ob[:, :], pb[DIM_OUT : 2 * DIM_OUT, :])
            del banks[(xi, h)]
            if h == 1:
                out_eng.dma_start(
                    out=out_d[:, xi * 512 : (xi + 1) * 512], in_=obs[xi][:, :]
                )
                del obs[xi]

        # groups xi = 0..32; group 32 contributes only rows 64-127 (out 30)
        for blk in range(GRID // 2 + 1):  # blocks of up to 2 plane-groups
            gxs = [g for g in (2 * blk, 2 * blk + 1) if g <= GRID]
            for g in gxs:
                for h in (0, 1):
                    banks[(g, h)] = ps_pool.tile(
                        [2 * DIM_OUT, 512], f32, name=f"bank_{g}_{h}", tag="bank"
                    )
            # weight-major inner order: w = 0: DR ky=0 (start), 1: center fp16,
            # 2: DR ky=1, 3: DR ky=2 (stop; last full-region write of the bank)
            for w in range(4):
                for xi in gxs:
                    if w == 1 and (xi == GRID or skip_center):
                        continue  # center reads zero pad plane 33; out 32 n/a
                    for h in (0, 1):
                        y0 = h * 16
                        ps = banks[(xi, h)]
                        if w == 1:
                            off = (y0 + 1) * GRID
                            rhs = xim_t[
                                2 * DIM_IN :, xi + 1 : xi + 2, off : off + 512
                            ].broadcast_to((2 * DIM_IN, 2, 512))
                            nc.tensor.matmul(
                                out=ps[0:DIM_OUT, :],
                                lhsT=wc3_t[2 * DIM_IN :, :, :],
                                rhs=rhs,
                                start=False,
                                stop=False,
                                perf_mode=DR,
                            )
                        else:
                            ky = {0: 0, 2: 1, 3: 2}[w]
                            off = (y0 + ky) * GRID
                            nc.tensor.matmul(
                                out=ps[:, :],
                                lhsT=wk3_t[:, 2 * ky : 2 * ky + 2, :],
                                rhs=xim_t[0:KPART, xi : xi + 2, off : off + 512],
                                start=(w == 0),
                                stop=(w == 3),
                                perf_mode=DR,
                            )
            # planes 2*blk-2 and 2*blk-1 are now complete
            for g in gxs:
                xr = g - 2
                if 0 <= xr < GRID:
                    for h in (0, 1):
                        evac(xr, h)
        for h in (0, 1):
            evac(GRID - 1, h)
            del banks[(GRID, h)]
        assert not banks, f"unevacuated banks: {list(banks)}"


# --------------------------------------------------------------------------
# runner
# --------------------------------------------------------------------------
def _get_program():
    if "nc" not in _PROGRAM_CACHE:
        _PROGRAM_CACHE["nc"] = build_program()
    return _PROGRAM_CACHE["nc"]


def kernel(x, w_lin0, w_lin1, w000, w011, w101, w110):
    from concourse.bass_utils import run_bass_kernel_spmd

    x = np.asarray(x, np.float32)
    k = build_conv_kernel(
        np.asarray(w_lin0), np.asarray(w_lin1),
        np.asarray(w000), np.asarray(w011), np.asarray(w101), np.asarray(w110),
    )
    wk3, wc3 = pack_weights(k)

    in_maps = []
    for b in range(BATCH):
        in_maps.append({"xim": build_im2col(x[b]), "wk3": wk3, "wc3": wc3})

    nc = _get_program()
    res = run_bass_kernel_spmd(nc, in_maps, list(range(N_CORES)))

    out = np.empty((BATCH, GRID, GRID, GRID, DIM_OUT), np.float32)
    for b in range(BATCH):
        out[b] = gather_out(res.results[b]["out"])
    return out


# revision 19
# speedup vs baseline: 2.0063x; 2.0063x over previous
"""Trainium2 Bass kernel for the e3nn-style 3D convolution problem.

Host side: builds the tiny [3,3,3,32,64] conv kernel from the radial/spherical
weights (replicating the reference math in fp32 numpy), folds the pointwise
self-connection into the center tap, and pre-arranges the input as a z-im2col
(3 z-shifted copies x 32 channels = 96 partitions) padded volume per batch.

Device side (per core, batch-parallel over 8 cores): 3D conv as accumulated
matmuls. Quantization split: everything except the center tap k[1,1,1] runs as
fp8(e4m3) DoubleRow matmuls — each DoubleRow instruction contracts TWO 96-row
windows at once (k-tile 0 = dx-paired taps (kx=0|kx=2) at base plane xi,
k-tile 1 = center-column singles (kx=1) at plane xi+1, same ky window — the
two k-tiles are adjacent planes of a 3D [96, 34, 1088] im2col tile, so the
rhs AP is a plain 3D slice). The dominant center tap (with the folded
self-connection) runs as one small fp16 matmul (K=32) per output tile from a
separate fp16 copy of the unshifted input. 4 matmuls per plane-group/half
instead of 6 full-rate ones; PSUM accumulation with cross-plane deferred
evacuation; output staged+DMAd as fp16 and upcast on host.
"""

import math

import numpy as np
import ml_dtypes

# ---- problem constants (hardcoded; kernel.py must be self-contained) ----
MUL_IN, MUL_OUT = 8, 16
DIM_IN, DIM_OUT = 4 * MUL_IN, 4 * MUL_OUT  # 32, 64
DIAMETER = 3.0
NUM_RB = 4
BATCH, GRID = 8, 32
N_CORES = 8

XP = GRID + 2  # padded x planes: -1 .. 32
YP = GRID + 2  # padded y rows
PLANE = YP * GRID  # floats per (padded-y, z) plane = 34*32 = 1088
KPART = 3 * DIM_IN  # 96 partitions: z-shift blocks (dz=-1,0,+1) x 32 channels
OUT_COLS = GRID * 512  # out dram [128, 16384]

FP8 = ml_dtypes.float8_e4m3  # TRN FP8_EXP4: bias 7, max normal +-240


# --------------------------------------------------------------------------
# host-side math: replicate the reference kernel build in fp32 numpy
# --------------------------------------------------------------------------
def _sus(x):
    # smooth unit step: exp(-1/x) for x>0 else 0
    safe = np.where(x > 0.0, x, 1.0).astype(np.float32)
    return np.where(x > 0.0, np.exp(np.float32(-1.0) / safe), np.float32(0.0))


def build_conv_kernel(w_lin0, w_lin1, w000, w011, w101, w110):
    """Returns K [3,3,3,DIM_IN,DIM_OUT] fp32 with the self-connection folded
    into the center tap."""
    f32 = np.float32
    r = DIAMETER / 2
    ax = np.arange(-math.floor(r), math.floor(r) + 1.0, dtype=f32)  # [-1,0,1]
    lattice = np.stack(np.meshgrid(ax, ax, ax, indexing="ij"), axis=-1).astype(f32)

    dist = np.linalg.norm(lattice, axis=-1).astype(f32)  # [3,3,3]
    values = np.linspace(0.0, DIAMETER / 2, NUM_RB + 2, dtype=f32)
    step = values[1] - values[0]
    diff = (dist[..., None] - values[1:-1]) / step  # [3,3,3,4]
    emb = (f32(1.14136) * np.exp(f32(2.0)) * _sus(diff + 1.0) * _sus(1.0 - diff)).astype(f32)

    norm = np.linalg.norm(lattice, axis=-1, keepdims=True).astype(f32)
    unit = lattice / np.where(norm == 0.0, f32(1.0), norm)
    sh1 = (np.sqrt(f32(3.0)) * unit).astype(f32)  # [3,3,3,3]

    n_lat = 27

    def rad(w):
        # emb [3,3,3,4] x w [4,8,1,16] -> [3,3,3,8,16]
        return (np.einsum("xyzk,kuvw->xyzuw", emb, w.astype(f32)) / f32(n_lat)).astype(f32)

    r000, r011, r101, r110 = rad(w000), rad(w011), rad(w101), rad(w110)

    inv_s3 = f32(1.0 / math.sqrt(3.0))
    alpha = f32(1.0 / math.sqrt(2.0 * MUL_IN))

    k00 = (alpha * r000).astype(f32)  # [3,3,3,8,16]
    k01 = (alpha * inv_s3) * np.einsum("xyzuw,xyzm->xyzuwm", r011, sh1)
    k01 = k01.reshape(3, 3, 3, MUL_IN, 3 * MUL_OUT).astype(f32)
    k10 = (alpha * inv_s3) * np.einsum("xyzuw,xyzi->xyzuiw", r110, sh1)
    k10 = k10.reshape(3, 3, 3, 3 * MUL_IN, MUL_OUT).astype(f32)
    eye3 = np.eye(3, dtype=f32)
    k11 = (alpha * inv_s3) * np.einsum("xyzuw,im->xyzuiwm", r101, eye3)
    k11 = k11.reshape(3, 3, 3, 3 * MUL_IN, 3 * MUL_OUT).astype(f32)

    k = np.concatenate(
        [
            np.concatenate([k00, k01], axis=-1),
            np.concatenate([k10, k11], axis=-1),
        ],
        axis=-2,
    ).astype(f32)  # [3,3,3,32,64]

    # ---- self-connection folded into the center tap ----
    lin_norm = f32(1.0 / math.sqrt(MUL_IN))
    w_sc = np.zeros((DIM_IN, DIM_OUT), f32)
    w_sc[:MUL_IN, :MUL_OUT] = w_lin0.astype(f32) * lin_norm
    for i in range(3):
        rows = MUL_IN + 3 * np.arange(MUL_IN) + i
        cols = MUL_OUT + 3 * np.arange(MUL_OUT) + i
        w_sc[np.ix_(rows, cols)] = w_lin1.astype(f32) * lin_norm
    k[1, 1, 1] += w_sc
    return k


def q8(a):
    return np.asarray(np.clip(a, -240.0, 240.0), FP8)


def pack_weights(k):
    """[3,3,3,32,64] -> (wk3 [96, 6, 128] fp8, wc3 [128, 2, 64] fp8).

    Contraction row blocks (matching build_im2col): 0-31 dz=-1, 32-63 dz=+1,
    64-95 dz=0.
    wk3[:, 2*ky+0, :]    = [k[0,ky] | k[2,ky]]   (dx-pair, base plane xi)
    wk3[:, 2*ky+1, 0:64] = k[1,ky]               (center column, plane xi+1),
                           with the dz=0 block of ky=1 (the true center tap,
                           incl. folded self-connection) zeroed -> moved to
                           the hi/lo-split center DR weights wc3:
    wc3[64:128, 0] = [W_hi; W_hi], wc3[64:128, 1] = [W_lo; 0]  (x_hi/x_lo
    partition blocks), so the center DR computes
    W_hi*(x_hi+x_lo) + W_lo*x_hi = W*x - W_lo*x_lo.
    """
    perm = np.r_[0:32, 64:96, 32:64]  # (dz-1, dz0, dz+1) -> (dz-1, dz+1, dz0)
    wk3 = np.zeros((KPART, 6, 2 * DIM_OUT), np.float32)
    for ky in range(3):
        wk3[:, 2 * ky, 0:DIM_OUT] = k[0, ky].reshape(KPART, DIM_OUT)[perm]
        wk3[:, 2 * ky, DIM_OUT:] = k[2, ky].reshape(KPART, DIM_OUT)[perm]
        wk3[:, 2 * ky + 1, 0:DIM_OUT] = k[1, ky].reshape(KPART, DIM_OUT)[perm]
    # remove the center tap from the windowed fp8 path (rows 64-95 = dz0)
    wk3[2 * DIM_IN : 3 * DIM_IN, 3, 0:DIM_OUT] = 0.0

    # center DR weights, padded to the same (128, 128) PE tile geometry as
    # the main DRs (differing tile_size/position between matmuls costs ~1us
    # of PE reconfiguration each)
    W = k[1, 1, 1].astype(np.float32)
    W_hi = q8(W).astype(np.float32)
    W_lo = W - W_hi
    wc3 = np.zeros((4 * DIM_IN, 2, 2 * DIM_OUT), np.float32)
    wc3[2 * DIM_IN : 3 * DIM_IN, 0, 0:DIM_OUT] = W_hi  # x_hi rows
    wc3[3 * DIM_IN :, 0, 0:DIM_OUT] = W_hi  # x_lo rows
    wc3[2 * DIM_IN : 3 * DIM_IN, 1, 0:DIM_OUT] = W_lo
    return q8(wk3), q8(wc3)


def build_im2col(xb):
    """xb [32,32,32,32] (X,Y,Z,C) -> xim [128,34,1088] fp8.

    Block 32*j + c layout: j=0: x[.,.,z-1,c] (dz=-1), j=1: x[.,.,z+1,c]
    (dz=+1), j=2: x_hi = fp8(x) (dz=0), j=3: x_lo = fp8(x - x_hi), all laid
    out as [xp 0..33][yp 0..33][z 0..31] with zero padding at xp/yp borders
    and z-shift edges."""
    xt = np.ascontiguousarray(xb.transpose(3, 0, 1, 2))  # [C, X, Y, Z]
    xim = np.zeros((4 * DIM_IN, XP, YP, GRID), np.float32)
    xim[0:32, 1:33, 1:33, 1:32] = xt[:, :, :, 0:31]  # dz=-1
    xim[32:64, 1:33, 1:33, 0:31] = xt[:, :, :, 1:32]  # dz=+1
    xim[64:96, 1:33, 1:33, :] = xt  # dz=0 -> x_hi
    x8 = q8(xim).reshape(4 * DIM_IN, XP, PLANE)
    lo = xim[64:96] - x8[64:96].astype(np.float32).reshape(DIM_IN, XP, YP, GRID)
    x8[96:128] = q8(lo).reshape(DIM_IN, XP, PLANE)
    # duplicate each plane (dup axis) so the center DR's two k-tiles can
    # read the same window through REAL strides — a stride-0 broadcast AP
    # costs ~1.2us per matmul on HW
    x4 = np.empty((4 * DIM_IN, XP, 2, PLANE), FP8)
    x4[:, :, 0] = x8
    x4[:, :, 1] = x8
    return x4


def gather_out(arr):
    """arr [128, 16384] fp16 -> [32, 32, 32, 64] fp32.

    Row p = (h*64 + co); column = xi*512 + yi*32 + z."""
    a = arr.astype(np.float32).reshape(2, DIM_OUT, GRID, 16, GRID)  # [h,co,xi,yi,z]
    return np.ascontiguousarray(a.transpose(2, 0, 3, 4, 1)).reshape(GRID, GRID, GRID, DIM_OUT)


# --------------------------------------------------------------------------
# device program
# --------------------------------------------------------------------------
_PROGRAM_CACHE = {}


def declare_tensors(nc):
    import concourse.mybir as mybir

    return dict(
        xim=nc.dram_tensor("xim", [4 * DIM_IN, XP, 2, PLANE], mybir.dt.float8e4, kind="ExternalInput").ap(),
        wk3=nc.dram_tensor("wk3", [KPART, 6, 2 * DIM_OUT], mybir.dt.float8e4, kind="ExternalInput").ap(),
        wc3=nc.dram_tensor("wc3", [4 * DIM_IN, 2, 2 * DIM_OUT], mybir.dt.float8e4, kind="ExternalInput").ap(),
        out=nc.dram_tensor("out", [2 * DIM_OUT, OUT_COLS], mybir.dt.float16, kind="ExternalOutput").ap(),
    )


def build_program():
    import concourse.tile as tile
    from concourse import bacc

    nc = bacc.Bacc(
        "TRN2",
        target_bir_lowering=False,
        debug=False,
        enable_asserts=True,
        num_devices=N_CORES,
    )
    T = declare_tensors(nc)
    with tile.TileContext(nc) as tc:
        emit_body(nc, tc, T)

    nc.compile()
    return nc


def emit_body(nc, tc, T):
    """fp8-DoubleRow scheme with per-plane [128, 512] fp16 output staging.

    For plane group xi (0..32), half h: psum bank (xi, h):
      rows 0-63   = out plane xi   (kx=0 pair-half + kx=1 singles + fp16 center)
      rows 64-127 = out plane xi-2 (kx=2 pair-half)
    Per bank 4 matmuls: DR(ky=0, start) -> center fp16 -> DR(ky=1)
    -> DR(ky=2, stop; full-region LAST so PSUM reads depend on the bank's
    final matmul — partial-region last writers race DVE evac reads on HW).
    Evacuation of plane xi: ob[h*64:(h+1)*64] = bank[xi,h][0:64]
    (+ bank[xi+2,h][64:128]), then one [128, 512] fp16 DMA per plane.
    """
    import concourse.mybir as mybir

    f32 = mybir.dt.float32
    f16 = mybir.dt.float16
    f8 = mybir.dt.float8e4
    DR = mybir.MatmulPerfMode.DoubleRow

    IN_CHUNKS = globals().get("IN_CHUNKS_OVR", 8)
    XC_CHUNKS = globals().get("XC_CHUNKS_OVR", 4)
    ob_bufs = globals().get("OB_BUFS_OVR", 6)
    out_eng = getattr(nc, globals().get("OUT_ENGINE", "scalar"))
    in_eng = getattr(nc, globals().get("IN_ENGINE", "sync"))
    xc_eng = getattr(nc, globals().get("XC_ENGINE", "gpsimd"))

    xim_d, wk3_d, wc3_d, out_d = T["xim"], T["wk3"], T["wc3"], T["out"]
    obdt = f32 if globals().get("OB_DTYPE") == "float32" else f16
    skip_center = globals().get("SKIP_CENTER", False)

    with (
        tc.tile_pool(name="xim", bufs=1) as xim_pool,
        tc.tile_pool(name="wk", bufs=1) as wk_pool,
        tc.tile_pool(name="ob", bufs=ob_bufs) as ob_pool,
        tc.tile_pool(name="ps", bufs=8, space="PSUM") as ps_pool,
    ):
        wk3_t = wk_pool.tile([KPART, 6, 2 * DIM_OUT], f8)
        nc.sync.dma_start(out=wk3_t[...], in_=wk3_d[...])
        wc3_t = wk_pool.tile([4 * DIM_IN, 2, 2 * DIM_OUT], f8)
        nc.sync.dma_start(out=wc3_t[...], in_=wc3_d[...])

        xim_t = xim_pool.tile([4 * DIM_IN, XP, 2, PLANE], f8)
        # chunked so matmuls can start once their planes have landed
        # (Tile tracks subtile deps); x_lo block rides on a second queue
        for ci in range(IN_CHUNKS):
            lo = ci * XP // IN_CHUNKS
            hi = (ci + 1) * XP // IN_CHUNKS
            in_eng.dma_start(out=xim_t[0:KPART, lo:hi, :, :], in_=xim_d[0:KPART, lo:hi, :, :])
        for ci in range(XC_CHUNKS):
            lo = ci * XP // XC_CHUNKS
            hi = (ci + 1) * XP // XC_CHUNKS
            xc_eng.dma_start(
                out=xim_t[KPART:, lo:hi, :, :], in_=xim_d[KPART:, lo:hi, :, :]
            )

        banks = {}
        obs = {}

        def evac(xi, h):
            if xi not in obs:
                obs[xi] = ob_pool.tile([2 * DIM_OUT, 512], obdt, name=f"ob_{xi}", tag="ob")
            ob = obs[xi][h * DIM_OUT : (h + 1) * DIM_OUT, :]
            pa = banks[(xi, h)]
            nc.scalar.copy(ob[:, :], pa[0:DIM_OUT, :])
            if xi < GRID - 1:
                pb = banks[(xi + 2, h)]
                nc.vector.tensor_add(ob[:, :], ob[:, :], pb[DIM_OUT : 2 * DIM_OUT, :])
            del banks[(xi, h)]
            if h == 1:
                out_eng.dma_start(
                    out=out_d[:, xi * 512 : (xi + 1) * 512], in_=obs[xi][:, :]
                )
                del obs[xi]

        # groups xi = 0..32; group 32 contributes only rows 64-127 (out 30)
        for blk in range(GRID // 2 + 1):  # blocks of up to 2 plane-groups
            gxs = [g for g in (2 * blk, 2 * blk + 1) if g <= GRID]
            for g in gxs:
                for h in (0, 1):
                    banks[(g, h)] = ps_pool.tile(
                        [2 * DIM_OUT, 512], f32, name=f"bank_{g}_{h}", tag="bank"
                    )
            # weight-major inner order: w = 0: DR ky=0 (start), 1: center fp16,
            # 2: DR ky=1, 3: DR ky=2 (stop; last full-region write of the bank)
            for w in range(4):
                for xi in gxs:
                    if w == 1 and (xi == GRID or skip_center):
                        continue  # center reads zero pad plane 33; out 32 n/a
                    for h in (0, 1):
                        y0 = h * 16
                        ps = banks[(xi, h)]
                        if w == 1:
                            off = (y0 + 1) * GRID
                            rhs = xim_t[
                                :, xi + 1 : xi + 2, :, off : off + 512
                            ].rearrange("p a b n -> p (a b) n")
                            nc.tensor.matmul(
                                out=ps[:, :],
                                lhsT=wc3_t[:, :, :],
                                rhs=rhs,
                                start=False,
                                stop=False,
                                perf_mode=DR,
                            )
                        else:
                            ky = {0: 0, 2: 1, 3: 2}[w]
                            off = (y0 + ky) * GRID
                            nc.tensor.matmul(
                                out=ps[:, :],
                                lhsT=wk3_t[:, 2 * ky : 2 * ky + 2, :],
                                rhs=xim_t[0:KPART, xi : xi + 2, 0:1, off : off + 512].rearrange(
                                    "p a b n -> p (a b) n"
                                ),
                                start=(w == 0),
                                stop=(w == 3),
                                perf_mode=DR,
                            )
            # planes 2*blk-2 and 2*blk-1 are now complete
            for g in gxs:
                xr = g - 2
                if 0 <= xr < GRID:
                    for h in (0, 1):
                        evac(xr, h)
        for h in (0, 1):
            evac(GRID - 1, h)
            del banks[(GRID, h)]
        assert not banks, f"unevacuated banks: {list(banks)}"


# --------------------------------------------------------------------------
# runner
# --------------------------------------------------------------------------
def _get_program():
    if "nc" not in _PROGRAM_CACHE:
        _PROGRAM_CACHE["nc"] = build_program()
    return _PROGRAM_CACHE["nc"]


def kernel(x, w_lin0, w_lin1, w000, w011, w101, w110):
    from concourse.bass_utils import run_bass_kernel_spmd

    x = np.asarray(x, np.float32)
    k = build_conv_kernel(
        np.asarray(w_lin0), np.asarray(w_lin1),
        np.asarray(w000), np.asarray(w011), np.asarray(w101), np.asarray(w110),
    )
    wk3, wc3 = pack_weights(k)

    in_maps = []
    for b in range(BATCH):
        in_maps.append({"xim": build_im2col(x[b]), "wk3": wk3, "wc3": wc3})

    nc = _get_program()
    res = run_bass_kernel_spmd(nc, in_maps, list(range(N_CORES)))

    out = np.empty((BATCH, GRID, GRID, GRID, DIM_OUT), np.float32)
    for b in range(BATCH):
        out[b] = gather_out(res.results[b]["out"])
    return out


# revision 22
# speedup vs baseline: 4.3072x; 2.1468x over previous
"""Trainium2 Bass kernel for the e3nn-style 3D convolution problem.

Host side: builds the tiny [3,3,3,32,64] conv kernel from the radial/spherical
weights (replicating the reference math in fp32 numpy), folds the pointwise
self-connection into the center tap, and pre-arranges the input as a z-im2col
(3 z-shifted copies x 32 channels = 96 partitions) padded volume per batch.

Device side (per core, batch-parallel over 8 cores): 3D conv as accumulated
matmuls. Quantization split: everything except the center tap k[1,1,1] runs as
fp8(e4m3) DoubleRow matmuls — each DoubleRow instruction contracts TWO 96-row
windows at once (k-tile 0 = dx-paired taps (kx=0|kx=2) at base plane xi,
k-tile 1 = center-column singles (kx=1) at plane xi+1, same ky window — the
two k-tiles are adjacent planes of a 3D [96, 34, 1088] im2col tile, so the
rhs AP is a plain 3D slice). The dominant center tap (with the folded
self-connection) runs as one small fp16 matmul (K=32) per output tile from a
separate fp16 copy of the unshifted input. 4 matmuls per plane-group/half
instead of 6 full-rate ones; PSUM accumulation with cross-plane deferred
evacuation; output staged+DMAd as fp16 and upcast on host.
"""

import math

import numpy as np
import ml_dtypes

# ---- problem constants (hardcoded; kernel.py must be self-contained) ----
MUL_IN, MUL_OUT = 8, 16
DIM_IN, DIM_OUT = 4 * MUL_IN, 4 * MUL_OUT  # 32, 64
DIAMETER = 3.0
NUM_RB = 4
BATCH, GRID = 8, 32
N_CORES = 8

XP = GRID + 2  # padded x planes: -1 .. 32
YP = GRID + 2  # padded y rows
PLANE = YP * GRID  # floats per (padded-y, z) plane = 34*32 = 1088
KPART = 3 * DIM_IN  # 96 partitions: z-shift blocks (dz=-1,0,+1) x 32 channels
OUT_COLS = GRID * 512  # out dram [128, 16384]

FP8 = ml_dtypes.float8_e4m3  # TRN FP8_EXP4: bias 7, max normal +-240


# --------------------------------------------------------------------------
# host-side math: replicate the reference kernel build in fp32 numpy
# --------------------------------------------------------------------------
def _sus(x):
    # smooth unit step: exp(-1/x) for x>0 else 0
    safe = np.where(x > 0.0, x, 1.0).astype(np.float32)
    return np.where(x > 0.0, np.exp(np.float32(-1.0) / safe), np.float32(0.0))


def build_conv_kernel(w_lin0, w_lin1, w000, w011, w101, w110):
    """Returns K [3,3,3,DIM_IN,DIM_OUT] fp32 with the self-connection folded
    into the center tap."""
    f32 = np.float32
    r = DIAMETER / 2
    ax = np.arange(-math.floor(r), math.floor(r) + 1.0, dtype=f32)  # [-1,0,1]
    lattice = np.stack(np.meshgrid(ax, ax, ax, indexing="ij"), axis=-1).astype(f32)

    dist = np.linalg.norm(lattice, axis=-1).astype(f32)  # [3,3,3]
    values = np.linspace(0.0, DIAMETER / 2, NUM_RB + 2, dtype=f32)
    step = values[1] - values[0]
    diff = (dist[..., None] - values[1:-1]) / step  # [3,3,3,4]
    emb = (f32(1.14136) * np.exp(f32(2.0)) * _sus(diff + 1.0) * _sus(1.0 - diff)).astype(f32)

    norm = np.linalg.norm(lattice, axis=-1, keepdims=True).astype(f32)
    unit = lattice / np.where(norm == 0.0, f32(1.0), norm)
    sh1 = (np.sqrt(f32(3.0)) * unit).astype(f32)  # [3,3,3,3]

    n_lat = 27

    def rad(w):
        # emb [3,3,3,4] x w [4,8,1,16] -> [3,3,3,8,16]
        return (np.einsum("xyzk,kuvw->xyzuw", emb, w.astype(f32)) / f32(n_lat)).astype(f32)

    r000, r011, r101, r110 = rad(w000), rad(w011), rad(w101), rad(w110)

    inv_s3 = f32(1.0 / math.sqrt(3.0))
    alpha = f32(1.0 / math.sqrt(2.0 * MUL_IN))

    k00 = (alpha * r000).astype(f32)  # [3,3,3,8,16]
    k01 = (alpha * inv_s3) * np.einsum("xyzuw,xyzm->xyzuwm", r011, sh1)
    k01 = k01.reshape(3, 3, 3, MUL_IN, 3 * MUL_OUT).astype(f32)
    k10 = (alpha * inv_s3) * np.einsum("xyzuw,xyzi->xyzuiw", r110, sh1)
    k10 = k10.reshape(3, 3, 3, 3 * MUL_IN, MUL_OUT).astype(f32)
    eye3 = np.eye(3, dtype=f32)
    k11 = (alpha * inv_s3) * np.einsum("xyzuw,im->xyzuiwm", r101, eye3)
    k11 = k11.reshape(3, 3, 3, 3 * MUL_IN, 3 * MUL_OUT).astype(f32)

    k = np.concatenate(
        [
            np.concatenate([k00, k01], axis=-1),
            np.concatenate([k10, k11], axis=-1),
        ],
        axis=-2,
    ).astype(f32)  # [3,3,3,32,64]

    # ---- self-connection folded into the center tap ----
    lin_norm = f32(1.0 / math.sqrt(MUL_IN))
    w_sc = np.zeros((DIM_IN, DIM_OUT), f32)
    w_sc[:MUL_IN, :MUL_OUT] = w_lin0.astype(f32) * lin_norm
    for i in range(3):
        rows = MUL_IN + 3 * np.arange(MUL_IN) + i
        cols = MUL_OUT + 3 * np.arange(MUL_OUT) + i
        w_sc[np.ix_(rows, cols)] = w_lin1.astype(f32) * lin_norm
    k[1, 1, 1] += w_sc
    return k


def q8(a):
    return np.asarray(np.clip(a, -240.0, 240.0), FP8)


def pack_weights(k):
    """[3,3,3,32,64] -> (wk3 [96, 6, 128] fp8, wc3 [128, 2, 64] fp8).

    Contraction row blocks (matching build_im2col): 0-31 dz=-1, 32-63 dz=+1,
    64-95 dz=0.
    wk3[:, 2*ky+0, :]    = [k[0,ky] | k[2,ky]]   (dx-pair, base plane xi)
    wk3[:, 2*ky+1, 0:64] = k[1,ky]               (center column, plane xi+1),
                           with the dz=0 block of ky=1 (the true center tap,
                           incl. folded self-connection) zeroed -> moved to
                           the hi/lo-split center DR weights wc3:
    wc3[64:128, 0] = [W_hi; W_hi], wc3[64:128, 1] = [W_lo; 0]  (x_hi/x_lo
    partition blocks), so the center DR computes
    W_hi*(x_hi+x_lo) + W_lo*x_hi = W*x - W_lo*x_lo.
    """
    perm = np.r_[0:32, 64:96, 32:64]  # (dz-1, dz0, dz+1) -> (dz-1, dz+1, dz0)
    P4 = 4 * DIM_IN
    wk3 = np.zeros((P4, 6, 2 * DIM_OUT), np.float32)
    for ky in range(3):
        wk3[0:KPART, 2 * ky, 0:DIM_OUT] = k[0, ky].reshape(KPART, DIM_OUT)[perm]
        wk3[0:KPART, 2 * ky, DIM_OUT:] = k[2, ky].reshape(KPART, DIM_OUT)[perm]
        wk3[0:KPART, 2 * ky + 1, 0:DIM_OUT] = k[1, ky].reshape(KPART, DIM_OUT)[perm]
    # the dz=0 block of the ky=1 singles IS the center tap; q8() rounds it to
    # W_hi. x_lo rows (96-127) of that same k-tile also get W_hi, so DR-ky1
    # contributes W_hi*x_hi + W_hi*x_lo. The remaining W_lo*x_hi term goes in
    # the dedicated center DR (wc3), whose 2nd k-tile is all-zero and reads
    # the next plane purely to keep real AP strides and uniform PE geometry.
    W = k[1, 1, 1].astype(np.float32)
    W_hi = q8(W).astype(np.float32)
    W_lo = W - W_hi
    wk3[3 * DIM_IN :, 3, 0:DIM_OUT] = W_hi
    wc3 = np.zeros((P4, 2, 2 * DIM_OUT), np.float32)
    wc3[2 * DIM_IN : 3 * DIM_IN, 0, 0:DIM_OUT] = W_lo  # x_hi rows, k-tile 0
    return q8(wk3), q8(wc3)


def build_im2col(xb):
    """xb [32,32,32,32] (X,Y,Z,C) -> xim [128,34,1088] fp8.

    Block 32*j + c layout: j=0: x[.,.,z-1,c] (dz=-1), j=1: x[.,.,z+1,c]
    (dz=+1), j=2: x_hi = fp8(x) (dz=0), j=3: x_lo = fp8(x - x_hi), all laid
    out as [xp 0..33][yp 0..33][z 0..31] with zero padding at xp/yp borders
    and z-shift edges."""
    xt = np.ascontiguousarray(xb.transpose(3, 0, 1, 2))  # [C, X, Y, Z]
    xim = np.zeros((4 * DIM_IN, XP, YP, GRID), np.float32)
    xim[0:32, 1:33, 1:33, 1:32] = xt[:, :, :, 0:31]  # dz=-1
    xim[32:64, 1:33, 1:33, 0:31] = xt[:, :, :, 1:32]  # dz=+1
    xim[64:96, 1:33, 1:33, :] = xt  # dz=0 -> x_hi
    x8 = q8(xim).reshape(4 * DIM_IN, XP, PLANE)
    lo = xim[64:96] - x8[64:96].astype(np.float32).reshape(DIM_IN, XP, YP, GRID)
    x8[96:128] = q8(lo).reshape(DIM_IN, XP, PLANE)
    return x8


def gather_out(arr):
    """arr [128, 16384] fp16 -> [32, 32, 32, 64] fp32.

    Row p = (h*64 + co); column = xi*512 + yi*32 + z."""
    a = arr.astype(np.float32).reshape(2, DIM_OUT, GRID, 16, GRID)  # [h,co,xi,yi,z]
    return np.ascontiguousarray(a.transpose(2, 0, 3, 4, 1)).reshape(GRID, GRID, GRID, DIM_OUT)


# --------------------------------------------------------------------------
# device program
# --------------------------------------------------------------------------
_PROGRAM_CACHE = {}


def declare_tensors(nc):
    import concourse.mybir as mybir

    return dict(
        xim=nc.dram_tensor("xim", [4 * DIM_IN, XP, PLANE], mybir.dt.float8e4, kind="ExternalInput").ap(),
        wk3=nc.dram_tensor("wk3", [4 * DIM_IN, 6, 2 * DIM_OUT], mybir.dt.float8e4, kind="ExternalInput").ap(),
        wc3=nc.dram_tensor("wc3", [4 * DIM_IN, 2, 2 * DIM_OUT], mybir.dt.float8e4, kind="ExternalInput").ap(),
        out=nc.dram_tensor("out", [2 * DIM_OUT, OUT_COLS], mybir.dt.float16, kind="ExternalOutput").ap(),
    )


def build_program():
    import concourse.tile as tile
    from concourse import bacc

    nc = bacc.Bacc(
        "TRN2",
        target_bir_lowering=False,
        debug=False,
        enable_asserts=True,
        num_devices=N_CORES,
    )
    T = declare_tensors(nc)
    with tile.TileContext(nc) as tc:
        emit_body(nc, tc, T)

    nc.compile()
    return nc


def emit_body(nc, tc, T):
    """fp8-DoubleRow scheme with per-plane [128, 512] fp16 output staging.

    For plane group xi (0..32), half h: psum bank (xi, h):
      rows 0-63   = out plane xi   (kx=0 pair-half + kx=1 singles + fp16 center)
      rows 64-127 = out plane xi-2 (kx=2 pair-half)
    Per bank 4 matmuls: DR(ky=0, start) -> center fp16 -> DR(ky=1)
    -> DR(ky=2, stop; full-region LAST so PSUM reads depend on the bank's
    final matmul — partial-region last writers race DVE evac reads on HW).
    Evacuation of plane xi: ob[h*64:(h+1)*64] = bank[xi,h][0:64]
    (+ bank[xi+2,h][64:128]), then one [128, 512] fp16 DMA per plane.
    """
    import concourse.mybir as mybir

    f32 = mybir.dt.float32
    f16 = mybir.dt.float16
    f8 = mybir.dt.float8e4
    DR = mybir.MatmulPerfMode.DoubleRow

    IN_CHUNKS = globals().get("IN_CHUNKS_OVR", 8)
    XC_CHUNKS = globals().get("XC_CHUNKS_OVR", 4)
    ob_bufs = globals().get("OB_BUFS_OVR", 6)
    out_eng = getattr(nc, globals().get("OUT_ENGINE", "scalar"))
    in_eng = getattr(nc, globals().get("IN_ENGINE", "sync"))
    xc_eng = getattr(nc, globals().get("XC_ENGINE", "gpsimd"))

    xim_d, wk3_d, wc3_d, out_d = T["xim"], T["wk3"], T["wc3"], T["out"]
    obdt = f32 if globals().get("OB_DTYPE") == "float32" else f16
    skip_center = globals().get("SKIP_CENTER", False)

    with (
        tc.tile_pool(name="xim", bufs=1) as xim_pool,
        tc.tile_pool(name="wk", bufs=1) as wk_pool,
        tc.tile_pool(name="ob", bufs=ob_bufs) as ob_pool,
        tc.tile_pool(name="ps", bufs=8, space="PSUM") as ps_pool,
    ):
        wk3_t = wk_pool.tile([4 * DIM_IN, 6, 2 * DIM_OUT], f8)
        nc.sync.dma_start(out=wk3_t[...], in_=wk3_d[...])
        wc3_t = wk_pool.tile([4 * DIM_IN, 2, 2 * DIM_OUT], f8)
        nc.sync.dma_start(out=wc3_t[...], in_=wc3_d[...])

        xim_t = xim_pool.tile([4 * DIM_IN, XP, PLANE], f8)
        # chunked so matmuls can start once their planes have landed
        # (Tile tracks subtile deps)
        for ci in range(IN_CHUNKS):
            lo = ci * XP // IN_CHUNKS
            hi = (ci + 1) * XP // IN_CHUNKS
            in_eng.dma_start(out=xim_t[:, lo:hi, :], in_=xim_d[:, lo:hi, :])

        banks = {}
        obs = {}

        def evac(xi, h):
            if xi not in obs:
                obs[xi] = ob_pool.tile([2 * DIM_OUT, 512], obdt, name=f"ob_{xi}", tag="ob")
            ob = obs[xi][h * DIM_OUT : (h + 1) * DIM_OUT, :]
            pa = banks[(xi, h)]
            nc.scalar.copy(ob[:, :], pa[0:DIM_OUT, :])
            if xi < GRID - 1:
                pb = banks[(xi + 2, h)]
                nc.vector.tensor_add(ob[:, :], ob[:, :], pb[DIM_OUT : 2 * DIM_OUT, :])
            del banks[(xi, h)]
            if h == 1:
                out_eng.dma_start(
                    out=out_d[:, xi * 512 : (xi + 1) * 512], in_=obs[xi][:, :]
                )
                del obs[xi]

        # groups xi = 0..32; group 32 contributes only rows 64-127 (out 30)
        for blk in range(GRID // 2 + 1):  # blocks of up to 2 plane-groups
            gxs = [g for g in (2 * blk, 2 * blk + 1) if g <= GRID]
            for g in gxs:
                for h in (0, 1):
                    banks[(g, h)] = ps_pool.tile(
                        [2 * DIM_OUT, 512], f32, name=f"bank_{g}_{h}", tag="bank"
                    )
            # weight-major inner order: w = 0: DR ky=0 (start), 1: center fp16,
            # 2: DR ky=1, 3: DR ky=2 (stop; last full-region write of the bank)
            for w in range(4):
                for xi in gxs:
                    if w == 1 and (xi == GRID or skip_center):
                        continue  # center reads zero pad plane 33; out 32 n/a
                    for h in (0, 1):
                        y0 = h * 16
                        ps = banks[(xi, h)]
                        if w == 1:
                            off = (y0 + 1) * GRID
                            rhs = xim_t[:, xi + 1 : xi + 3, off : off + 512]
                            nc.tensor.matmul(
                                out=ps[:, :],
                                lhsT=wc3_t[:, :, :],
                                rhs=rhs,
                                start=False,
                                stop=False,
                                perf_mode=DR,
                            )
                        else:
                            ky = {0: 0, 2: 1, 3: 2}[w]
                            off = (y0 + ky) * GRID
                            nc.tensor.matmul(
                                out=ps[:, :],
                                lhsT=wk3_t[:, 2 * ky : 2 * ky + 2, :],
                                rhs=xim_t[:, xi : xi + 2, off : off + 512],
                                start=(w == 0),
                                stop=(w == 3),
                                perf_mode=DR,
                            )
            # planes 2*blk-2 and 2*blk-1 are now complete
            for g in gxs:
                xr = g - 2
                if 0 <= xr < GRID:
                    for h in (0, 1):
                        evac(xr, h)
        for h in (0, 1):
            evac(GRID - 1, h)
            del banks[(GRID, h)]
        assert not banks, f"unevacuated banks: {list(banks)}"


# --------------------------------------------------------------------------
# runner
# --------------------------------------------------------------------------
def _get_program():
    if "nc" not in _PROGRAM_CACHE:
        _PROGRAM_CACHE["nc"] = build_program()
    return _PROGRAM_CACHE["nc"]


def kernel(x, w_lin0, w_lin1, w000, w011, w101, w110):
    from concourse.bass_utils import run_bass_kernel_spmd

    x = np.asarray(x, np.float32)
    k = build_conv_kernel(
        np.asarray(w_lin0), np.asarray(w_lin1),
        np.asarray(w000), np.asarray(w011), np.asarray(w101), np.asarray(w110),
    )
    wk3, wc3 = pack_weights(k)

    in_maps = []
    for b in range(BATCH):
        in_maps.append({"xim": build_im2col(x[b]), "wk3": wk3, "wc3": wc3})

    nc = _get_program()
    res = run_bass_kernel_spmd(nc, in_maps, list(range(N_CORES)))

    out = np.empty((BATCH, GRID, GRID, GRID, DIM_OUT), np.float32)
    for b in range(BATCH):
        out[b] = gather_out(res.results[b]["out"])
    return out


# revision 23
# speedup vs baseline: 12.9143x; 2.9983x over previous
"""Trainium2 Bass kernel for the e3nn-style 3D convolution problem.

Host side: builds the tiny [3,3,3,32,64] conv kernel from the radial/spherical
weights (replicating the reference math in fp32 numpy), folds the pointwise
self-connection into the center tap, and pre-arranges the input as a z-im2col
(3 z-shifted copies x 32 channels = 96 partitions) padded volume per batch.

Device side (per core, batch-parallel over 8 cores): 3D conv as accumulated
matmuls. Quantization split: everything except the center tap k[1,1,1] runs as
fp8(e4m3) DoubleRow matmuls — each DoubleRow instruction contracts TWO 96-row
windows at once (k-tile 0 = dx-paired taps (kx=0|kx=2) at base plane xi,
k-tile 1 = center-column singles (kx=1) at plane xi+1, same ky window — the
two k-tiles are adjacent planes of a 3D [96, 34, 1088] im2col tile, so the
rhs AP is a plain 3D slice). The dominant center tap (with the folded
self-connection) runs as one small fp16 matmul (K=32) per output tile from a
separate fp16 copy of the unshifted input. 4 matmuls per plane-group/half
instead of 6 full-rate ones; PSUM accumulation with cross-plane deferred
evacuation; output staged+DMAd as fp16 and upcast on host.
"""

import math

import numpy as np
import ml_dtypes

# ---- problem constants (hardcoded; kernel.py must be self-contained) ----
MUL_IN, MUL_OUT = 8, 16
DIM_IN, DIM_OUT = 4 * MUL_IN, 4 * MUL_OUT  # 32, 64
DIAMETER = 3.0
NUM_RB = 4
BATCH, GRID = 8, 32
N_CORES = 8

XP = GRID + 2  # padded x planes: -1 .. 32
YP = GRID + 2  # padded y rows
PLANE = YP * GRID  # floats per (padded-y, z) plane = 34*32 = 1088
KPART = 3 * DIM_IN  # 96 partitions: z-shift blocks (dz=-1,0,+1) x 32 channels
OUT_COLS = GRID * 512  # out dram [128, 16384]

FP8 = ml_dtypes.float8_e4m3  # TRN FP8_EXP4: bias 7, max normal +-240


# --------------------------------------------------------------------------
# host-side math: replicate the reference kernel build in fp32 numpy
# --------------------------------------------------------------------------
def _sus(x):
    # smooth unit step: exp(-1/x) for x>0 else 0
    safe = np.where(x > 0.0, x, 1.0).astype(np.float32)
    return np.where(x > 0.0, np.exp(np.float32(-1.0) / safe), np.float32(0.0))


def build_conv_kernel(w_lin0, w_lin1, w000, w011, w101, w110):
    """Returns K [3,3,3,DIM_IN,DIM_OUT] fp32 with the self-connection folded
    into the center tap."""
    f32 = np.float32
    r = DIAMETER / 2
    ax = np.arange(-math.floor(r), math.floor(r) + 1.0, dtype=f32)  # [-1,0,1]
    lattice = np.stack(np.meshgrid(ax, ax, ax, indexing="ij"), axis=-1).astype(f32)

    dist = np.linalg.norm(lattice, axis=-1).astype(f32)  # [3,3,3]
    values = np.linspace(0.0, DIAMETER / 2, NUM_RB + 2, dtype=f32)
    step = values[1] - values[0]
    diff = (dist[..., None] - values[1:-1]) / step  # [3,3,3,4]
    emb = (f32(1.14136) * np.exp(f32(2.0)) * _sus(diff + 1.0) * _sus(1.0 - diff)).astype(f32)

    norm = np.linalg.norm(lattice, axis=-1, keepdims=True).astype(f32)
    unit = lattice / np.where(norm == 0.0, f32(1.0), norm)
    sh1 = (np.sqrt(f32(3.0)) * unit).astype(f32)  # [3,3,3,3]

    n_lat = 27

    def rad(w):
        # emb [3,3,3,4] x w [4,8,1,16] -> [3,3,3,8,16]
        return (np.einsum("xyzk,kuvw->xyzuw", emb, w.astype(f32)) / f32(n_lat)).astype(f32)

    r000, r011, r101, r110 = rad(w000), rad(w011), rad(w101), rad(w110)

    inv_s3 = f32(1.0 / math.sqrt(3.0))
    alpha = f32(1.0 / math.sqrt(2.0 * MUL_IN))

    k00 = (alpha * r000).astype(f32)  # [3,3,3,8,16]
    k01 = (alpha * inv_s3) * np.einsum("xyzuw,xyzm->xyzuwm", r011, sh1)
    k01 = k01.reshape(3, 3, 3, MUL_IN, 3 * MUL_OUT).astype(f32)
    k10 = (alpha * inv_s3) * np.einsum("xyzuw,xyzi->xyzuiw", r110, sh1)
    k10 = k10.reshape(3, 3, 3, 3 * MUL_IN, MUL_OUT).astype(f32)
    eye3 = np.eye(3, dtype=f32)
    k11 = (alpha * inv_s3) * np.einsum("xyzuw,im->xyzuiwm", r101, eye3)
    k11 = k11.reshape(3, 3, 3, 3 * MUL_IN, 3 * MUL_OUT).astype(f32)

    k = np.concatenate(
        [
            np.concatenate([k00, k01], axis=-1),
            np.concatenate([k10, k11], axis=-1),
        ],
        axis=-2,
    ).astype(f32)  # [3,3,3,32,64]

    # ---- self-connection folded into the center tap ----
    lin_norm = f32(1.0 / math.sqrt(MUL_IN))
    w_sc = np.zeros((DIM_IN, DIM_OUT), f32)
    w_sc[:MUL_IN, :MUL_OUT] = w_lin0.astype(f32) * lin_norm
    for i in range(3):
        rows = MUL_IN + 3 * np.arange(MUL_IN) + i
        cols = MUL_OUT + 3 * np.arange(MUL_OUT) + i
        w_sc[np.ix_(rows, cols)] = w_lin1.astype(f32) * lin_norm
    k[1, 1, 1] += w_sc
    return k


def q8(a):
    return np.asarray(np.clip(a, -240.0, 240.0), FP8)


def pack_weights(k):
    """[3,3,3,32,64] -> (wk3 [96, 6, 128] fp8, wc3 [128, 2, 64] fp8).

    Contraction row blocks (matching build_im2col): 0-31 dz=-1, 32-63 dz=+1,
    64-95 dz=0.
    wk3[:, 2*ky+0, :]    = [k[0,ky] | k[2,ky]]   (dx-pair, base plane xi)
    wk3[:, 2*ky+1, 0:64] = k[1,ky]               (center column, plane xi+1),
                           with the dz=0 block of ky=1 (the true center tap,
                           incl. folded self-connection) zeroed -> moved to
                           the hi/lo-split center DR weights wc3:
    wc3[64:128, 0] = [W_hi; W_hi], wc3[64:128, 1] = [W_lo; 0]  (x_hi/x_lo
    partition blocks), so the center DR computes
    W_hi*(x_hi+x_lo) + W_lo*x_hi = W*x - W_lo*x_lo.
    """
    perm = np.r_[0:32, 64:96, 32:64]  # (dz-1, dz0, dz+1) -> (dz-1, dz+1, dz0)
    P4 = 4 * DIM_IN
    wk3 = np.zeros((P4, 6, 2 * DIM_OUT), np.float32)
    for ky in range(3):
        wk3[0:KPART, 2 * ky, 0:DIM_OUT] = k[0, ky].reshape(KPART, DIM_OUT)[perm]
        wk3[0:KPART, 2 * ky, DIM_OUT:] = k[2, ky].reshape(KPART, DIM_OUT)[perm]
        wk3[0:KPART, 2 * ky + 1, 0:DIM_OUT] = k[1, ky].reshape(KPART, DIM_OUT)[perm]
    # the dz=0 block of the ky=1 singles IS the center tap; q8() rounds it to
    # W_hi. x_lo rows (96-127) of that same k-tile also get W_hi, so DR-ky1
    # contributes W_hi*x_hi + W_hi*x_lo. The remaining W_lo*x_hi term goes in
    # the dedicated center DR (wc3), whose 2nd k-tile is all-zero and reads
    # the next plane purely to keep real AP strides and uniform PE geometry.
    W = k[1, 1, 1].astype(np.float32)
    W_hi = q8(W).astype(np.float32)
    W_lo = W - W_hi
    wk3[3 * DIM_IN :, 3, 0:DIM_OUT] = W_hi
    wc3 = np.zeros((P4, 2, 2 * DIM_OUT), np.float32)
    wc3[2 * DIM_IN : 3 * DIM_IN, 0, 0:DIM_OUT] = W_lo  # x_hi rows, k-tile 0
    return q8(wk3), q8(wc3)


def build_im2col(xb):
    """xb [32,32,32,32] (X,Y,Z,C) -> xim [128,34,1088] fp8.

    Block 32*j + c layout: j=0: x[.,.,z-1,c] (dz=-1), j=1: x[.,.,z+1,c]
    (dz=+1), j=2: x_hi = fp8(x) (dz=0), j=3: x_lo = fp8(x - x_hi), all laid
    out as [xp 0..33][yp 0..33][z 0..31] with zero padding at xp/yp borders
    and z-shift edges."""
    xt = np.ascontiguousarray(xb.transpose(3, 0, 1, 2))  # [C, X, Y, Z]
    xim = np.zeros((4 * DIM_IN, XP, YP, GRID), np.float32)
    xim[0:32, 1:33, 1:33, 1:32] = xt[:, :, :, 0:31]  # dz=-1
    xim[32:64, 1:33, 1:33, 0:31] = xt[:, :, :, 1:32]  # dz=+1
    xim[64:96, 1:33, 1:33, :] = xt  # dz=0 -> x_hi
    x8 = q8(xim).reshape(4 * DIM_IN, XP, PLANE)
    lo = xim[64:96] - x8[64:96].astype(np.float32).reshape(DIM_IN, XP, YP, GRID)
    x8[96:128] = q8(lo).reshape(DIM_IN, XP, PLANE)
    return x8


def gather_out(arr):
    """arr [128, 16384] fp16 -> [32, 32, 32, 64] fp32.

    Row p = (h*64 + co); column = xi*512 + yi*32 + z."""
    a = arr.astype(np.float32).reshape(2, DIM_OUT, GRID, 16, GRID)  # [h,co,xi,yi,z]
    return np.ascontiguousarray(a.transpose(2, 0, 3, 4, 1)).reshape(GRID, GRID, GRID, DIM_OUT)


# --------------------------------------------------------------------------
# device program
# --------------------------------------------------------------------------
_PROGRAM_CACHE = {}


def declare_tensors(nc):
    import concourse.mybir as mybir

    return dict(
        xim=nc.dram_tensor("xim", [4 * DIM_IN, XP, PLANE], mybir.dt.float8e4, kind="ExternalInput").ap(),
        wk3=nc.dram_tensor("wk3", [4 * DIM_IN, 6, 2 * DIM_OUT], mybir.dt.float8e4, kind="ExternalInput").ap(),
        wc3=nc.dram_tensor("wc3", [4 * DIM_IN, 2, 2 * DIM_OUT], mybir.dt.float8e4, kind="ExternalInput").ap(),
        out=nc.dram_tensor("out", [2 * DIM_OUT, OUT_COLS], mybir.dt.float16, kind="ExternalOutput").ap(),
    )


def build_program():
    import concourse.tile as tile
    from concourse import bacc

    nc = bacc.Bacc(
        "TRN2",
        target_bir_lowering=False,
        debug=False,
        enable_asserts=True,
        num_devices=N_CORES,
    )
    T = declare_tensors(nc)
    with tile.TileContext(nc) as tc:
        emit_body(nc, tc, T)

    nc.compile()
    return nc


def emit_body(nc, tc, T):
    """fp8-DoubleRow scheme with per-plane [128, 512] fp16 output staging.

    For plane group xi (0..32), half h: psum bank (xi, h):
      rows 0-63   = out plane xi   (kx=0 pair-half + kx=1 singles + fp16 center)
      rows 64-127 = out plane xi-2 (kx=2 pair-half)
    Per bank 4 matmuls: DR(ky=0, start) -> center fp16 -> DR(ky=1)
    -> DR(ky=2, stop; full-region LAST so PSUM reads depend on the bank's
    final matmul — partial-region last writers race DVE evac reads on HW).
    Evacuation of plane xi: ob[h*64:(h+1)*64] = bank[xi,h][0:64]
    (+ bank[xi+2,h][64:128]), then one [128, 512] fp16 DMA per plane.
    """
    import concourse.mybir as mybir

    f32 = mybir.dt.float32
    f16 = mybir.dt.float16
    f8 = mybir.dt.float8e4
    DR = mybir.MatmulPerfMode.DoubleRow

    IN_CHUNKS = globals().get("IN_CHUNKS_OVR", 8)
    XC_CHUNKS = globals().get("XC_CHUNKS_OVR", 4)
    ob_bufs = globals().get("OB_BUFS_OVR", 6)
    out_eng = getattr(nc, globals().get("OUT_ENGINE", "scalar"))
    in_eng = getattr(nc, globals().get("IN_ENGINE", "sync"))
    xc_eng = getattr(nc, globals().get("XC_ENGINE", "gpsimd"))

    xim_d, wk3_d, wc3_d, out_d = T["xim"], T["wk3"], T["wc3"], T["out"]
    obdt = f32 if globals().get("OB_DTYPE") == "float32" else f16
    skip_center = globals().get("SKIP_CENTER", False)

    with (
        tc.tile_pool(name="xim", bufs=globals().get("XIM_BUFS_OVR", 1)) as xim_pool,
        tc.tile_pool(name="wk", bufs=1) as wk_pool,
        tc.tile_pool(name="ob", bufs=ob_bufs) as ob_pool,
        tc.tile_pool(name="ps", bufs=8, space="PSUM") as ps_pool,
    ):
        wk3_t = wk_pool.tile([4 * DIM_IN, 6, 2 * DIM_OUT], f8)
        nc.sync.dma_start(out=wk3_t[...], in_=wk3_d[...])
        wc3_t = wk_pool.tile([4 * DIM_IN, 2, 2 * DIM_OUT], f8)
        nc.sync.dma_start(out=wc3_t[...], in_=wc3_d[...])

        xim_t = xim_pool.tile([4 * DIM_IN, XP, PLANE], f8)
        # chunked so matmuls can start once their planes have landed
        # (Tile tracks subtile deps)
        for ci in range(IN_CHUNKS):
            lo = ci * XP // IN_CHUNKS
            hi = (ci + 1) * XP // IN_CHUNKS
            in_eng.dma_start(out=xim_t[:, lo:hi, :], in_=xim_d[:, lo:hi, :])

        banks = {}
        obs = {}

        def evac(xi, h):
            if xi not in obs:
                obs[xi] = ob_pool.tile([2 * DIM_OUT, 512], obdt, name=f"ob_{xi}", tag="ob")
            ob = obs[xi][h * DIM_OUT : (h + 1) * DIM_OUT, :]
            pa = banks[(xi, h)]
            nc.scalar.copy(ob[:, :], pa[0:DIM_OUT, :])
            if xi < GRID - 1:
                pb = banks[(xi + 2, h)]
                nc.vector.tensor_add(ob[:, :], ob[:, :], pb[DIM_OUT : 2 * DIM_OUT, :])
            del banks[(xi, h)]
            if h == 1:
                out_eng.dma_start(
                    out=out_d[:, xi * 512 : (xi + 1) * 512], in_=obs[xi][:, :]
                )
                del obs[xi]

        # groups xi = 0..32; group 32 contributes only rows 64-127 (out 30)
        for blk in range(GRID // 2 + 1):  # blocks of up to 2 plane-groups
            gxs = [g for g in (2 * blk, 2 * blk + 1) if g <= GRID]
            for g in gxs:
                for h in (0, 1):
                    banks[(g, h)] = ps_pool.tile(
                        [2 * DIM_OUT, 512], f32, name=f"bank_{g}_{h}", tag="bank"
                    )
            # weight-major inner order: w = 0: DR ky=0 (start), 1: center fp16,
            # 2: DR ky=1, 3: DR ky=2 (stop; last full-region write of the bank)
            for w in range(4):
                for xi in gxs:
                    if w == 1 and (xi == GRID or skip_center):
                        continue  # center reads zero pad plane 33; out 32 n/a
                    for h in (0, 1):
                        y0 = h * 16
                        ps = banks[(xi, h)]
                        if w == 1:
                            off = (y0 + 1) * GRID
                            rhs = xim_t[:, xi + 1 : xi + 3, off : off + 512]
                            nc.tensor.matmul(
                                out=ps[:, :],
                                lhsT=wc3_t[:, :, :],
                                rhs=rhs,
                                start=False,
                                stop=False,
                                perf_mode=DR,
                            )
                        else:
                            ky = {0: 0, 2: 1, 3: 2}[w]
                            off = (y0 + ky) * GRID
                            nc.tensor.matmul(
                                out=ps[:, :],
                                lhsT=wk3_t[:, 2 * ky : 2 * ky + 2, :],
                                rhs=xim_t[:, xi : xi + 2, off : off + 512],
                                start=(w == 0),
                                stop=(w == 3),
                                perf_mode=DR,
                            )
            # planes 2*blk-2 and 2*blk-1 are now complete
            for g in gxs:
                xr = g - 2
                if 0 <= xr < GRID:
                    for h in (0, 1):
                        evac(xr, h)
        for h in (0, 1):
            evac(GRID - 1, h)
            del banks[(GRID, h)]
        assert not banks, f"unevacuated banks: {list(banks)}"


# --------------------------------------------------------------------------
# runner
# --------------------------------------------------------------------------
def _get_program():
    if "nc" not in _PROGRAM_CACHE:
        _PROGRAM_CACHE["nc"] = build_program()
    return _PROGRAM_CACHE["nc"]


def kernel(x, w_lin0, w_lin1, w000, w011, w101, w110):
    from concourse.bass_utils import run_bass_kernel_spmd

    x = np.asarray(x, np.float32)
    k = build_conv_kernel(
        np.asarray(w_lin0), np.asarray(w_lin1),
        np.asarray(w000), np.asarray(w011), np.asarray(w101), np.asarray(w110),
    )
    wk3, wc3 = pack_weights(k)

    in_maps = []
    for b in range(BATCH):
        in_maps.append({"xim": build_im2col(x[b]), "wk3": wk3, "wc3": wc3})

    nc = _get_program()
    res = run_bass_kernel_spmd(nc, in_maps, list(range(N_CORES)))

    out = np.empty((BATCH, GRID, GRID, GRID, DIM_OUT), np.float32)
    for b in range(BATCH):
        out[b] = gather_out(res.results[b]["out"])
    return out
